# revision 11
# baseline (speedup 1.0000x reference)
"""BinarizedLinear on 8 Trainium2 NeuronCores.

out = x @ sign(weight).T + bias
  x: (32768, 1024) f32, weight: (1024, 1024) f32, bias: (1024,) f32

Strategy (data-parallel over batch, weight/bias replicated):
  - each core handles a 4096-row shard of x
  - host marshals the shard to bf16 in a [p, su, ic, b] tiled layout
    (p = feature % 128 -> SBUF partition, su = batch/128 tile, ic =
    feature/128 contraction chunk, b = batch % 128) so that each batch
    window is ONE DMA with multi-KB contiguous per-partition segments
    (large descriptors -> near-peak HBM bandwidth) while every matmul
    stationary tile xs[.., ic*128:+128] stays a contiguous
    256B-per-partition slice (fast weight load stays enabled)
  - the binarized +-1 weight is exact in fp8, host-packed [p, ic, o']
    per output half and shipped as one 512KB DMA per half; both halves plus the
    host-replicated bias lead the scalar queue ahead of the output
    stores, so the sync queue stays pure-x and window 0 lands at full
    rate
  - device: PE matmul (x tile stationary, K=1024 accumulated in PSUM
    over 8 chunks, N=512 free) -> DVE bias-add writing bf16 ->
    contiguous 256KB store (scalar queue)
  - output returned as bf16 [4096, 1024]; host upcasts to f32
  - x windows ramp 128..2048 batch rows, all enqueued up front on the
    sync queue so DMA runs far ahead of the PE; a warmup burst
    un-throttles the PE clock (HAM) during the first window's fill
"""

import os
import sys

import numpy as np

sys.path.insert(0, "/opt/trn_rl_repo")

import ml_dtypes

import concourse.tile as tile
from concourse import bacc, mybir
from concourse.bass_utils import run_bass_kernel_spmd

N_CORES = 8
B_FULL = 32768
I_DIM = 1024
O_DIM = 1024
BS = B_FULL // N_CORES  # 4096 batch rows per core

P = 128                # partitions / contraction tile
IC = I_DIM // P        # 8 contraction chunks
N_OC = 512             # psum free width (one PSUM bank of f32)
OC = O_DIM // N_OC     # 2 output chunks
B_SUB = 128            # stationary-operand free width (psum partitions)
N_SU = BS // B_SUB     # 32 batch tiles per core
WINDOWS = [1, 1, 2, 4, 8, 16]  # batch windows in su units
assert sum(WINDOWS) == N_SU
N_WARM = 7
SU_W = B_SUB * IC      # elements per su per partition (1024)

F32 = mybir.dt.float32
BF16 = mybir.dt.bfloat16
FP8 = mybir.dt.float8e4

_cache = {}


def _build_program():
    nc = bacc.Bacc("TRN2", target_bir_lowering=False, debug=False,
                   num_devices=N_CORES)

    xt = nc.dram_tensor("xt", [P, N_SU * SU_W], BF16,
                        kind="ExternalInput").ap()
    wt0 = nc.dram_tensor("wt0", [P, IC * N_OC], FP8,
                         kind="ExternalInput").ap()
    wt1 = nc.dram_tensor("wt1", [P, IC * N_OC], FP8,
                         kind="ExternalInput").ap()
    # bias pre-replicated across partitions on the host: a plain 512KB
    # line-rate DMA instead of a 128x4KB-packet broadcast (whose per-
    # packet overhead starves the concurrent x stream)
    bias_d = nc.dram_tensor("bias_d", [P, O_DIM], F32,
                            kind="ExternalInput").ap()
    out = nc.dram_tensor("out", [BS, O_DIM], BF16, kind="ExternalOutput").ap()

    with tile.TileContext(nc) as tc:
        with (
            tc.tile_pool(name="consts", bufs=1) as consts,
            tc.tile_pool(name="xb", bufs=1) as xb_pool,
            tc.tile_pool(name="ot", bufs=8) as ot_pool,
            tc.tile_pool(name="ps", bufs=6, space="PSUM") as ps_pool,
        ):
            # PE warmup: data-independent matmuls on scratch SBUF keep the
            # PE busy through the first window's DMA fill so HAM
            # un-throttles to 2.4 GHz before the first real matmul.
            warm_sc = consts.tile([P, N_OC], BF16)
            nc.vector.memset(warm_sc[:], 0.0)
            ps_w = ps_pool.tile([P, N_OC], F32, tag="warm", bufs=1)
            for _ in range(N_WARM):
                nc.tensor.matmul(ps_w[:], warm_sc[:, :B_SUB], warm_sc[:],
                                 start=True, stop=True, skip_group_check=True)

            # Weights + bias lead the scalar queue ahead of the stores; the
            # sync queue stays pure-x so window 0 lands at full rate.
            wt_sb = []
            for oc, wsrc in enumerate((wt0, wt1)):
                w_t = consts.tile([P, IC * N_OC], FP8, tag=f"wt{oc}")
                if oc == 0:
                    # two halves so group 0's first matmuls gate on 256KB
                    half = IC * N_OC // 2
                    nc.scalar.dma_start(w_t[:, :half], wsrc[:, :half])
                    nc.scalar.dma_start(w_t[:, half:], wsrc[:, half:])
                else:
                    nc.scalar.dma_start(w_t[:], wsrc[:, :])
                wt_sb.append(w_t)
            bias_sb = consts.tile([P, O_DIM], F32)
            nc.scalar.dma_start(bias_sb[:], bias_d[:, :])

            # x windows: one DMA each, enqueued up front on the sync queue.
            off = [0]
            for w in WINDOWS:
                off.append(off[-1] + w)
            xw = []
            for wi, w in enumerate(WINDOWS):
                s0 = off[wi]
                xs = xb_pool.tile([P, w * SU_W], BF16, tag=f"xs{wi}", bufs=1)
                nc.sync.dma_start(xs[:], xt[:, s0 * SU_W:(s0 + w) * SU_W])
                xw.append(xs)

            for wi, w in enumerate(WINDOWS):
                s0 = off[wi]
                for lsu in range(w):
                    su = s0 + lsu
                    r0 = su * B_SUB
                    last = su == N_SU - 1
                    ot = ot_pool.tile([P, O_DIM], BF16, tag="ot")
                    for oc in range(OC):
                        ps = ps_pool.tile([P, N_OC], F32, tag="ps")
                        for k in range(IC):
                            nc.tensor.matmul(
                                ps[:],
                                xw[wi][:, lsu * SU_W + k * B_SUB:
                                       lsu * SU_W + k * B_SUB + B_SUB],
                                wt_sb[oc][:, k * N_OC:(k + 1) * N_OC],
                                start=(k == 0),
                                stop=(k == IC - 1),
                            )
                        nc.vector.tensor_add(
                            ot[:, oc * N_OC:(oc + 1) * N_OC], ps[:],
                            bias_sb[:, oc * N_OC:(oc + 1) * N_OC])
                        if last:
                            # tail: ship each half as soon as it's ready
                            nc.scalar.dma_start(
                                out[r0:r0 + B_SUB,
                                    oc * N_OC:(oc + 1) * N_OC],
                                ot[:, oc * N_OC:(oc + 1) * N_OC])
                    if not last:
                        # 256KB fully-contiguous bf16 store of 128 rows.
                        nc.scalar.dma_start(out[r0:r0 + B_SUB, :], ot[:])

    nc.compile()
    return nc


def _get_program():
    if "prog" not in _cache:
        _cache["prog"] = _build_program()
    return _cache["prog"]


def _marshal_w(weight: np.ndarray):
    s = np.sign(weight)
    s[s == 0] = 1.0
    w3 = s.T.reshape(IC, P, O_DIM)  # [ic, p, o]
    halves = []
    for oc in range(OC):
        h = np.ascontiguousarray(
            w3[:, :, oc * N_OC:(oc + 1) * N_OC].transpose(1, 0, 2))
        halves.append(h.reshape(P, IC * N_OC).astype(ml_dtypes.float8_e4m3))
    return halves


def _marshal_x(x_shard: np.ndarray) -> np.ndarray:
    # [B, F] -> [su, b, ic, p] -> [p, su, ic, b], bf16
    x4 = x_shard.reshape(N_SU, B_SUB, IC, P).transpose(3, 0, 2, 1)
    return np.ascontiguousarray(x4).astype(ml_dtypes.bfloat16).reshape(
        P, N_SU * SU_W)


def kernel_impl(x, weight, bias, mode=None, trace=False, tmpdir=None):
    wt0, wt1 = _marshal_w(np.asarray(weight))
    bias_d = np.ascontiguousarray(
        np.broadcast_to(np.asarray(bias, np.float32)[None, :], (P, O_DIM)))
    x = np.asarray(x, np.float32)

    in_maps = []
    for c in range(N_CORES):
        in_maps.append({"xt": _marshal_x(x[c * BS:(c + 1) * BS]),
                        "wt0": wt0, "wt1": wt1, "bias_d": bias_d})

    nc = _get_program()
    try:
        res = run_bass_kernel_spmd(nc, in_maps, list(range(N_CORES)),
                                   trace=trace, tmpdir=tmpdir)
    except Exception:
        # transient runtime hiccups (e.g. first dispatch after long idle)
        res = run_bass_kernel_spmd(nc, in_maps, list(range(N_CORES)),
                                   trace=trace, tmpdir=tmpdir)
    out = np.concatenate(
        [np.asarray(res.results[c]["out"]).astype(np.float32)
         for c in range(N_CORES)], axis=0)
    return out, res


def kernel(x, weight, bias):
    out, _ = kernel_impl(x, weight, bias)
    return out


# revision 15
# speedup vs baseline: 1.2229x; 1.2229x over previous
"""BinarizedLinear on 8 Trainium2 NeuronCores.

out = x @ sign(weight).T + bias
  x: (32768, 1024) f32, weight: (1024, 1024) f32, bias: (1024,) f32

Strategy (data-parallel over batch, weight/bias replicated):
  - each core handles a 4096-row shard of x
  - host marshals the shard to bf16 in a [p, su, ic, b] tiled layout
    (p = feature % 128 -> SBUF partition, su = batch/128 tile, ic =
    feature/128 contraction chunk, b = batch % 128) so that each batch
    window is ONE DMA with multi-KB contiguous per-partition segments
    (large descriptors -> near-peak HBM bandwidth) while every matmul
    stationary tile xs[.., ic*128:+128] stays a contiguous
    256B-per-partition slice (fast weight load stays enabled)
  - the binarized +-1 weight is exact in fp8, host-packed [p, ic, o']
    per output half and shipped as one 512KB DMA per half; both halves plus the
    host-replicated bias lead the scalar queue ahead of the output
    stores, so the sync queue stays pure-x and window 0 lands at full
    rate
  - device: PE matmul (x tile stationary, K=1024 accumulated in PSUM
    over 8 chunks, N=512 free) -> DVE bias-add writing bf16 ->
    contiguous 256KB store (scalar queue)
  - output returned as bf16 [4096, 1024]; host upcasts to f32
  - x windows ramp 128..2048 batch rows, all enqueued up front on the
    sync queue so DMA runs far ahead of the PE; a warmup burst
    un-throttles the PE clock (HAM) during the first window's fill
"""

import os
import sys

import numpy as np

sys.path.insert(0, "/opt/trn_rl_repo")

import ml_dtypes

import concourse.tile as tile
from concourse import bacc, mybir
from concourse.bass_utils import run_bass_kernel_spmd

N_CORES = 8
B_FULL = 32768
I_DIM = 1024
O_DIM = 1024
BS = B_FULL // N_CORES  # 4096 batch rows per core

P = 128                # partitions / contraction tile
IC = I_DIM // P        # 8 contraction chunks
N_OC = 512             # psum free width (one PSUM bank of f32)
OC = O_DIM // N_OC     # 2 output chunks
B_SUB = 128            # stationary-operand free width (psum partitions)
N_SU = BS // B_SUB     # 32 batch tiles per core
WINDOWS = [1, 1, 2, 4, 8, 16]  # batch windows in su units
assert sum(WINDOWS) == N_SU
N_WARM = 7
SU_W = B_SUB * IC      # elements per su per partition (1024)

F32 = mybir.dt.float32
BF16 = mybir.dt.bfloat16
FP8 = mybir.dt.float8e4

_cache = {}


def _build_program():
    nc = bacc.Bacc("TRN2", target_bir_lowering=False, debug=False,
                   num_devices=N_CORES)

    xt = nc.dram_tensor("xt", [P, N_SU * SU_W], BF16,
                        kind="ExternalInput").ap()
    wt0 = nc.dram_tensor("wt0", [P, IC * N_OC], FP8,
                         kind="ExternalInput").ap()
    wt1 = nc.dram_tensor("wt1", [P, IC * N_OC], FP8,
                         kind="ExternalInput").ap()
    # bias pre-replicated across partitions on the host: a plain 512KB
    # line-rate DMA instead of a 128x4KB-packet broadcast (whose per-
    # packet overhead starves the concurrent x stream)
    bias_d = nc.dram_tensor("bias_d", [P, O_DIM], F32,
                            kind="ExternalInput").ap()
    out = nc.dram_tensor("out", [BS, O_DIM], BF16, kind="ExternalOutput").ap()

    with tile.TileContext(nc) as tc:
        with (
            tc.tile_pool(name="consts", bufs=1) as consts,
            tc.tile_pool(name="xb", bufs=1) as xb_pool,
            tc.tile_pool(name="ot", bufs=8) as ot_pool,
            tc.tile_pool(name="ps", bufs=6, space="PSUM") as ps_pool,
        ):
            # PE warmup: data-independent matmuls on scratch SBUF keep the
            # PE busy through the first window's DMA fill so HAM
            # un-throttles to 2.4 GHz before the first real matmul.
            warm_sc = consts.tile([P, N_OC], BF16)
            nc.vector.memset(warm_sc[:], 0.0)
            ps_w = ps_pool.tile([P, N_OC], F32, tag="warm", bufs=1)
            for _ in range(N_WARM):
                nc.tensor.matmul(ps_w[:], warm_sc[:, :B_SUB], warm_sc[:],
                                 start=True, stop=True, skip_group_check=True)

            # Weights + bias lead the scalar queue ahead of the stores; the
            # sync queue stays pure-x so window 0 lands at full rate.
            # oc0 weights split into two tiles so group 0's first matmuls
            # gate on just 256KB; the k0-3 half rides the sync queue head
            # (starts ~1us before scalar). Separate tiles keep the two
            # halves' DMA dependencies fully independent.
            half = IC * N_OC // 2
            wt0a = consts.tile([P, half], FP8, tag="wt0a")
            nc.sync.dma_start(wt0a[:], wt0[:, :half])
            wt0b = consts.tile([P, half], FP8, tag="wt0b")
            nc.scalar.dma_start(wt0b[:], wt0[:, half:])
            wt1_sb = consts.tile([P, IC * N_OC], FP8, tag="wt1")
            nc.scalar.dma_start(wt1_sb[:], wt1[:, :])

            def w_slice(oc, k):
                if oc == 1:
                    return wt1_sb[:, k * N_OC:(k + 1) * N_OC]
                if k < IC // 2:
                    return wt0a[:, k * N_OC:(k + 1) * N_OC]
                kk = k - IC // 2
                return wt0b[:, kk * N_OC:(kk + 1) * N_OC]
            bias_sb = consts.tile([P, O_DIM], F32)
            nc.scalar.dma_start(bias_sb[:], bias_d[:, :])

            # x windows: one DMA each, enqueued up front on the sync queue.
            off = [0]
            for w in WINDOWS:
                off.append(off[-1] + w)
            xw = []
            for wi, w in enumerate(WINDOWS):
                s0 = off[wi]
                xs = xb_pool.tile([P, w * SU_W], BF16, tag=f"xs{wi}", bufs=1)
                nc.sync.dma_start(xs[:], xt[:, s0 * SU_W:(s0 + w) * SU_W])
                xw.append(xs)

            for wi, w in enumerate(WINDOWS):
                s0 = off[wi]
                for lsu in range(w):
                    su = s0 + lsu
                    r0 = su * B_SUB
                    last = su == N_SU - 1
                    ot = ot_pool.tile([P, O_DIM], BF16, tag="ot")
                    for oc in range(OC):
                        ps = ps_pool.tile([P, N_OC], F32, tag="ps")
                        for k in range(IC):
                            nc.tensor.matmul(
                                ps[:],
                                xw[wi][:, lsu * SU_W + k * B_SUB:
                                       lsu * SU_W + k * B_SUB + B_SUB],
                                w_slice(oc, k),
                                start=(k == 0),
                                stop=(k == IC - 1),
                            )
                        nc.vector.tensor_add(
                            ot[:, oc * N_OC:(oc + 1) * N_OC], ps[:],
                            bias_sb[:, oc * N_OC:(oc + 1) * N_OC])
                        if last:
                            # tail: ship each half as soon as it's ready
                            nc.scalar.dma_start(
                                out[r0:r0 + B_SUB,
                                    oc * N_OC:(oc + 1) * N_OC],
                                ot[:, oc * N_OC:(oc + 1) * N_OC])
                    if not last:
                        # 256KB fully-contiguous bf16 store of 128 rows.
                        nc.scalar.dma_start(out[r0:r0 + B_SUB, :], ot[:])

    nc.compile()
    return nc


def _get_program():
    if "prog" not in _cache:
        _cache["prog"] = _build_program()
    return _cache["prog"]


def _marshal_w(weight: np.ndarray):
    s = np.sign(weight)
    s[s == 0] = 1.0
    w3 = s.T.reshape(IC, P, O_DIM)  # [ic, p, o]
    halves = []
    for oc in range(OC):
        h = np.ascontiguousarray(
            w3[:, :, oc * N_OC:(oc + 1) * N_OC].transpose(1, 0, 2))
        halves.append(h.reshape(P, IC * N_OC).astype(ml_dtypes.float8_e4m3))
    return halves


def _marshal_x(x_shard: np.ndarray) -> np.ndarray:
    # [B, F] -> [su, b, ic, p] -> [p, su, ic, b], bf16
    x4 = x_shard.reshape(N_SU, B_SUB, IC, P).transpose(3, 0, 2, 1)
    return np.ascontiguousarray(x4).astype(ml_dtypes.bfloat16).reshape(
        P, N_SU * SU_W)


def kernel_impl(x, weight, bias, mode=None, trace=False, tmpdir=None):
    wt0, wt1 = _marshal_w(np.asarray(weight))
    bias_d = np.ascontiguousarray(
        np.broadcast_to(np.asarray(bias, np.float32)[None, :], (P, O_DIM)))
    x = np.asarray(x, np.float32)

    in_maps = []
    for c in range(N_CORES):
        in_maps.append({"xt": _marshal_x(x[c * BS:(c + 1) * BS]),
                        "wt0": wt0, "wt1": wt1, "bias_d": bias_d})

    nc = _get_program()
    try:
        res = run_bass_kernel_spmd(nc, in_maps, list(range(N_CORES)),
                                   trace=trace, tmpdir=tmpdir)
    except Exception:
        # transient runtime hiccups (e.g. first dispatch after long idle)
        res = run_bass_kernel_spmd(nc, in_maps, list(range(N_CORES)),
                                   trace=trace, tmpdir=tmpdir)
    out = np.concatenate(
        [np.asarray(res.results[c]["out"]).astype(np.float32)
         for c in range(N_CORES)], axis=0)
    return out, res


def kernel(x, weight, bias):
    out, _ = kernel_impl(x, weight, bias)
    return out
